# revision 18
# baseline (speedup 1.0000x reference)
"""Multi-head causal self-attention (B=2, T=4096, D=768, H=12) on 8 trn2 cores.

Sharding: core c -> batch b = c//4, heads 3*(c%4) .. 3*(c%4)+2.
qkv_proj column-parallel (each core computes Q/K/V only for its heads),
out_proj row-parallel (each core emits a partial y^T; host sums the 4
partials per batch).

v3: bf16 matmuls + host-side x^T; software-pipelined schedule: projection
and out-proj matmuls are injected as PE "filler" work between attention
k-pair steps so the tensor engine never idles (keeps the DVFS p-state at
max); causal band tiles trim their invalid columns from the scores/exp/AV
work (single 128x128 triangle mask replaces the wide band masks); AV psum
accumulators are copied to SBUF immediately so their banks recycle fast.

Device dataflow:
  x^T bf16 DMA'd per 512-col chunk -> Q^T/K^T via transposed projection
  (W^T stationary, x^T streaming) -> V natural (x^T chunks stationary,
  Wv^T streaming) -> S^T = K Q^T in [k,q] layout, two heads row-paired on
  opposite PE halves -> exp on ScalarE writing bf16 -> triangle masks on
  DVE for diagonal tiles -> out^T = V^T P^T with a ones column appended
  to V collecting softmax denominators in psum row 64 -> normalize via
  batched reciprocal + gpsimd partition broadcast (double-buffered by qb
  parity) -> y^T = Wo^T.T out^T, deferred one q-block as filler work.
"""

import sys

sys.path.insert(0, "/opt/trn_rl_repo")

import numpy as np
import ml_dtypes
from collections import deque
from contextlib import ExitStack

import concourse.bass as bass
import concourse.bacc as bacc
import concourse.tile as tile
import concourse.mybir as mybir
from concourse.bass_utils import run_bass_kernel_spmd

F32 = mybir.dt.float32
BF16 = mybir.dt.bfloat16
F8 = mybir.dt.float8e4
NPBF = np.dtype(ml_dtypes.bfloat16)
AF = mybir.ActivationFunctionType
DROW = mybir.MatmulPerfMode.DoubleRow

B = 2
T = 4096
D = 768
H = 12
DK = 64
NCORES = 8
HL = 3  # heads per core
ND = D // 128  # 6 d-tiles
NKT = T // 128  # 32 k-tiles
NQB = T // 512  # 8 q-blocks
NTSB = T // 512  # 8 t-superblocks

_CACHE = {}


def _emit(tc):
    nc = tc.nc
    xT_d = nc.dram_tensor("xT", [D, T], BF16, kind="ExternalInput").ap()
    wqk_d = nc.dram_tensor("wqkT", [D, 384], BF16, kind="ExternalInput").ap()
    wv_d = nc.dram_tensor("wvT", [D, HL * DK], BF16, kind="ExternalInput").ap()
    wo_d = nc.dram_tensor("woT", [HL, DK, D], BF16, kind="ExternalInput").ap()
    y_d = nc.dram_tensor("yT", [D, T], F32, kind="ExternalOutput").ap()

    ctx = ExitStack()
    const = ctx.enter_context(tc.tile_pool(name="const", bufs=1))
    persist = ctx.enter_context(tc.tile_pool(name="persist", bufs=1))
    ptpool = ctx.enter_context(tc.tile_pool(name="pt", bufs=8))
    spool = ctx.enter_context(tc.tile_pool(name="sp", bufs=2))
    ypool = ctx.enter_context(tc.tile_pool(name="yp", bufs=2))
    # PSUM (8 banks): psA 2x[128,1024]f32 = 4 for score tiles; psB 2x1 for
    # AV accumulators; psC 2x1 shared by projection / out-proj fillers.
    psA = ctx.enter_context(tc.tile_pool(name="psA", bufs=2, space="PSUM"))
    psB = ctx.enter_context(tc.tile_pool(name="psB", bufs=2, space="PSUM"))
    psC = ctx.enter_context(tc.tile_pool(name="psC", bufs=2, space="PSUM"))

    # ---- constants ----
    # triangle mask for the first 128 valid columns of each diagonal band
    # tile: tri[k, j] = 1 for j >= k else 0
    tri = const.tile([128, 128], BF16, name="tri")
    nc.gpsimd.memset(tri, 1.0)
    nc.gpsimd.affine_select(
        out=tri, in_=tri, compare_op=mybir.AluOpType.is_ge, fill=0.0,
        base=0, pattern=[[1, 128]], channel_multiplier=-1,
    )

    wqk_sb = const.tile([128, ND, 384], BF16)
    nc.sync.dma_start(out=wqk_sb, in_=wqk_d.rearrange("(j p) e -> p j e", p=128))
    wv_sb = const.tile([128, ND, HL * DK], BF16)
    nc.sync.dma_start(out=wv_sb, in_=wv_d.rearrange("(j p) e -> p j e", p=128))
    wo01_sb = const.tile([128, D], BF16)  # head0 rows on 0:64, head1 on 64:128
    nc.sync.dma_start(out=wo01_sb, in_=wo_d[0:2].rearrange("h p d -> (h p) d"))
    wo2_sb = const.tile([DK, D], BF16)
    nc.sync.dma_start(out=wo2_sb, in_=wo_d[2])

    # ---- persistent activations ----
    xT_sb = persist.tile([128, ND, T], BF16, name="xT")
    # KA: [K^T_h0 ; K^T_h1], QB: [Q^T_h0 ; Q^T_h1] on partition halves
    KA = persist.tile([128, T], BF16, name="KA")
    QB = persist.tile([128, T], BF16, name="QB")
    C2 = persist.tile([128, T], BF16, name="C2")  # [K^T_h2 ; Q^T_h2]
    D2 = persist.tile([128, T], BF16, name="D2")  # [Q^T_h2 ; K^T_h2] (swapped)
    V = persist.tile([128, HL, NKT, DK + 1], BF16, name="V")
    nc.gpsimd.memset(V[:, :, :, DK : DK + 1], 1.0)  # ones col -> softmax sums
    # out^T staging, double-buffered by q-block parity (out-proj is deferred
    # into the next q-block's filler slots)
    ot01 = [persist.tile([128, 512], BF16, name=f"ot01_{p}") for p in range(2)]
    ot2 = [persist.tile([DK, 512], BF16, name=f"ot2_{p}") for p in range(2)]

    qk_dest = [KA, QB, C2]
    xT_dr = xT_d.rearrange("(j p) t -> p j t", p=128)

    # ---- filler items: projection work for one t-superblock ----
    def proj_fillers(tsb):
        blk = slice(tsb * 512, (tsb + 1) * 512)

        def dma_item():
            if tsb == 0:
                # split per d-tile so the first QK matmul chain can start
                # as soon as its first operand stripe lands
                for dj in range(ND):
                    nc.sync.dma_start(
                        out=xT_sb[:, dj, blk], in_=xT_dr[:, dj, blk]
                    )
            else:
                nc.sync.dma_start(out=xT_sb[:, :, blk], in_=xT_dr[:, :, blk])

        def qk_item(et):
            ps_q = psC.tile([128, 512], F32, name="ps_q", tag="pc")
            e0 = et * 128
            for dj in range(ND):
                nc.tensor.matmul(
                    ps_q,
                    lhsT=wqk_sb[:, dj, e0 : e0 + 128],
                    rhs=xT_sb[:, dj, blk],
                    start=(dj == 0), stop=(dj == ND - 1),
                )
            nc.vector.tensor_copy(qk_dest[et][:, blk], ps_q)
            if et == 2:
                # D2 = partition-swapped copy of C2 (h2 self-pairing)
                nc.sync.dma_start(out=D2[0:64, blk], in_=C2[64:128, blk])
                nc.sync.dma_start(out=D2[64:128, blk], in_=C2[0:64, blk])

        def v_item(tt):
            t0 = (tsb * 4 + tt) * 128
            ps_v = psC.tile([128, HL * DK], F32, name="ps_v", tag="pc")
            for dj in range(ND):
                nc.tensor.matmul(
                    ps_v,
                    lhsT=xT_sb[:, dj, t0 : t0 + 128],
                    rhs=wv_sb[:, dj, :],
                    start=(dj == 0), stop=(dj == ND - 1),
                )
            kt = tsb * 4 + tt
            nc.vector.tensor_copy(
                V[:, :, kt, 0:DK], ps_v.rearrange("p (h c) -> p h c", h=HL)
            )

        items = [dma_item]
        items += [lambda et=et: qk_item(et) for et in range(3)]
        items += [lambda tt=tt: v_item(tt) for tt in range(4)]
        return items

    # ---- filler items: out-projection of one q-block ----
    def outproj_fillers(qb):
        qblk = slice(qb * 512, (qb + 1) * 512)
        o01, o2 = ot01[qb % 2], ot2[qb % 2]

        def y_item(dj):
            dblk = slice(dj * 128, (dj + 1) * 128)
            ps_y = psC.tile([128, 512], F32, name="ps_y", tag="pc")
            nc.tensor.matmul(
                ps_y, lhsT=wo01_sb[:, dblk], rhs=o01,
                start=True, stop=False, skip_group_check=True,
            )
            nc.tensor.matmul(
                ps_y, lhsT=wo2_sb[:, dblk], rhs=o2,
                start=False, stop=True, skip_group_check=True,
            )
            y_sb = ypool.tile([128, 512], F32, name="y_sb")
            nc.vector.tensor_copy(y_sb, ps_y)
            nc.sync.dma_start(out=y_d[dblk, qblk], in_=y_sb)

        return [lambda dj=dj: y_item(dj) for dj in range(ND)]

    # fq_proj: hard deadline (drained before the attention block that reads
    # it); fq_out: deferred out-proj, drains opportunistically.
    fq_proj = deque()
    fq_out = deque()

    def emit_fillers(n):
        for _ in range(n):
            if fq_proj:
                fq_proj.popleft()()
            elif fq_out:
                fq_out.popleft()()
            else:
                return

    # warm-up: project t-superblock 0 before attention starts
    for it in proj_fillers(0):
        it()
    fq_proj.extend(proj_fillers(1))

    # ================= attention, pipelined =================
    for qb in range(NQB):
        nk = 4 * (qb + 1)
        o01, o2 = ot01[qb % 2], ot2[qb % 2]
        # k-pair list; the two diagonal-band pairs are reordered (hi, lo)
        # so the exp range stays a single contiguous span per pt tile
        pairs = [(2 * kp, 2 * kp + 1) for kp in range(nk // 2)]
        pairs[-2] = (nk - 3, nk - 4)
        pairs[-1] = (nk - 1, nk - 2)
        for hpass, heads in enumerate([(0, 1), (2,)]):
            psav = {h: psB.tile([DK + 1, 512], F32, name=f"psav{h}", tag="pb")
                    for h in heads}
            for kt_a, kt_b in pairs:
                ss = {h: psA.tile([128, 1024], F32, name=f"ss{h}", tag="pa")
                      for h in heads}
                lo_a = max(0, kt_a - 4 * qb) * 128  # valid col start (band)
                lo_b = max(0, kt_b - 4 * qb) * 128
                for i, (kt, lo) in enumerate(((kt_a, lo_a), (kt_b, lo_b))):
                    kblk = slice(kt * 128, (kt + 1) * 128)
                    dst = slice(i * 512 + lo, i * 512 + 512)
                    qsub = slice(qb * 512 + lo, (qb + 1) * 512)
                    if hpass == 0:
                        nc.tensor.matmul(
                            ss[0][:, dst], lhsT=KA[0:64, kblk],
                            rhs=QB[0:64, qsub], start=True, stop=True,
                        )
                        nc.tensor.matmul(
                            ss[1][:, dst], lhsT=KA[64:128, kblk],
                            rhs=QB[64:128, qsub], start=True, stop=True,
                        )
                    elif i == 0:
                        nc.tensor.matmul(
                            ss[2][:, dst], lhsT=C2[0:64, kblk],
                            rhs=D2[0:64, qsub], start=True, stop=True,
                        )
                    else:
                        nc.tensor.matmul(
                            ss[2][:, dst], lhsT=D2[64:128, kblk],
                            rhs=C2[64:128, qsub], start=True, stop=True,
                        )
                emit_fillers(1)
                band = kt_a >= 4 * qb
                for h in heads:
                    pt = ptpool.tile([128, 1024], BF16, name="pt")
                    # one contiguous exp span [lo_a:1024]; for a reordered
                    # band pair (hi, lo) any gap columns hold junk that the
                    # AV rhs slices below never touch
                    nc.scalar.activation(
                        pt[:, lo_a:1024], ss[h][:, lo_a:1024], AF.Exp,
                        scale=0.125,
                    )
                    # triangle mask on the first 128 valid cols of band tiles
                    if band:
                        for i, (kt, lo) in enumerate(((kt_a, lo_a), (kt_b, lo_b))):
                            c0 = i * 512 + lo
                            nc.vector.tensor_mul(
                                pt[:, c0 : c0 + 128], pt[:, c0 : c0 + 128], tri
                            )
                    # AV ascending kt within the pair (kt==0 carries the
                    # full-width start=True that initializes the bank)
                    for i, kt, lo in sorted(
                        ((0, kt_a, lo_a), (1, kt_b, lo_b)), key=lambda e: e[1]
                    ):
                        nc.tensor.matmul(
                            psav[h][:, lo:512],
                            lhsT=V[:, h, kt, :],
                            rhs=pt[:, i * 512 + lo : i * 512 + 512],
                            start=(kt == 0), stop=(kt == nk - 1),
                            skip_group_check=True,
                        )
                    emit_fillers(1)
                if len(fq_proj) + len(fq_out) > 10:
                    emit_fillers(1)
            # normalize: out^T = psav rows 0:64 / sums (row 64). DVE takes
            # the reciprocal of the psum sums row in place (partition 64),
            # gpsimd broadcasts it to 64 partitions (sbuf-to-sbuf).
            for h in heads:
                recipH = spool.tile(
                    [DK + 1, 512], F32, name="recipH", tag="rh"
                )[DK : DK + 1, :]
                nc.vector.reciprocal(recipH, psav[h][DK : DK + 1, :])
                recipR = spool.tile([1, 512], F32, name="recipR", tag="rr")
                nc.sync.dma_start(out=recipR, in_=recipH)
                recipb = spool.tile([DK, 512], F32, name="recipb", tag="rb")
                nc.gpsimd.partition_broadcast(recipb, recipR, channels=DK)
                if h == 0:
                    nc.vector.tensor_mul(o01[0:DK, :], psav[h][0:DK, :], recipb)
                elif h == 1:
                    ot1s = spool.tile([DK, 512], BF16, name="ot1s", tag="o1")
                    nc.vector.tensor_mul(ot1s, psav[h][0:DK, :], recipb)
                    nc.sync.dma_start(out=o01[DK:128, :], in_=ot1s)
                else:
                    nc.vector.tensor_mul(o2, psav[h][0:DK, :], recipb)
        # hard deadline: projections for the next q-block must be fully
        # emitted before its attention reads KA/QB/C2/D2/V
        emit_n = len(fq_proj)
        for _ in range(emit_n):
            fq_proj.popleft()()
        # defer this q-block's out-projection into upcoming filler slots
        fq_out.extend(outproj_fillers(qb))
        if qb + 2 < NTSB:
            fq_proj.extend(proj_fillers(qb + 2))
    while fq_out:
        fq_out.popleft()()
    ctx.close()


def build():
    if "nc" in _CACHE:
        return _CACHE["nc"]
    nc = bacc.Bacc(
        "TRN2", target_bir_lowering=False, debug=False, num_devices=NCORES
    )
    with tile.TileContext(nc) as tc:
        _emit(tc)
    nc.compile()
    _CACHE["nc"] = nc
    return nc


def make_in_maps(x, w_qkv, w_out):
    x = np.asarray(x, dtype=np.float32)
    w_qkv = np.asarray(w_qkv, dtype=np.float32)
    w_out = np.asarray(w_out, dtype=np.float32)
    wq = w_qkv[0:D]        # [768, 768], rows = q features
    wk = w_qkv[D : 2 * D]
    wv = w_qkv[2 * D :]
    xT = [np.ascontiguousarray(x[b].T).astype(NPBF) for b in range(B)]
    in_maps = []
    for c in range(NCORES):
        b, g = divmod(c, 4)
        hs = [3 * g + j for j in range(HL)]  # global head ids
        h0, h1, h2 = hs
        cols = []
        for pair in ((wk, h0), (wk, h1), (wq, h0), (wq, h1), (wk, h2), (wq, h2)):
            w, h = pair
            cols.append(w[h * DK : (h + 1) * DK].T)  # [768, 64]
        wqkT = np.ascontiguousarray(np.concatenate(cols, axis=1)).astype(NPBF)
        wvT = np.ascontiguousarray(
            np.concatenate([wv[h * DK : (h + 1) * DK].T for h in hs], axis=1)
        ).astype(NPBF)  # [768, 192]
        woT = np.ascontiguousarray(
            np.stack([w_out[:, h * DK : (h + 1) * DK].T for h in hs])
        ).astype(NPBF)  # [3, 64, 768]
        in_maps.append(
            {
                "xT": xT[b],
                "wqkT": wqkT,
                "wvT": wvT,
                "woT": woT,
            }
        )
    return in_maps


def run(inputs, trace=False):
    """Run on hardware; returns (y [B,T,D] fp32, BassKernelResults)."""
    nc = build()
    in_maps = make_in_maps(inputs["x"], inputs["w_qkv"], inputs["w_out"])
    br = run_bass_kernel_spmd(nc, in_maps, list(range(NCORES)), trace=trace)
    y = np.zeros((B, T, D), dtype=np.float32)
    for c in range(NCORES):
        b = c // 4
        y[b] += np.asarray(br.results[c]["yT"], dtype=np.float32).T
    return y, br


def kernel(x, w_qkv, w_out):
    y, _ = run({"x": x, "w_qkv": w_qkv, "w_out": w_out})
    return y


# revision 21
# speedup vs baseline: 1.2366x; 1.2366x over previous
"""Multi-head causal self-attention (B=2, T=4096, D=768, H=12) on 8 trn2 cores.

Sharding: core c -> batch b = c//4, heads 3*(c%4) .. 3*(c%4)+2.
qkv_proj column-parallel (each core computes Q/K/V only for its heads),
out_proj row-parallel (each core emits a partial y^T; host sums the 4
partials per batch).

v3: bf16 matmuls + host-side x^T; software-pipelined schedule: projection
and out-proj matmuls are injected as PE "filler" work between attention
k-pair steps so the tensor engine never idles (keeps the DVFS p-state at
max); causal band tiles trim their invalid columns from the scores/exp/AV
work (single 128x128 triangle mask replaces the wide band masks); AV psum
accumulators are copied to SBUF immediately so their banks recycle fast.

Device dataflow:
  x^T bf16 DMA'd per 512-col chunk -> Q^T/K^T via transposed projection
  (W^T stationary, x^T streaming) -> V natural (x^T chunks stationary,
  Wv^T streaming) -> S^T = K Q^T in [k,q] layout, two heads row-paired on
  opposite PE halves -> exp on ScalarE writing bf16 -> triangle masks on
  DVE for diagonal tiles -> out^T = V^T P^T with a ones column appended
  to V collecting softmax denominators in psum row 64 -> normalize via
  batched reciprocal + gpsimd partition broadcast (double-buffered by qb
  parity) -> y^T = Wo^T.T out^T, deferred one q-block as filler work.
"""

import sys

sys.path.insert(0, "/opt/trn_rl_repo")

import numpy as np
import ml_dtypes
from collections import deque
from contextlib import ExitStack

import concourse.bass as bass
import concourse.bacc as bacc
import concourse.tile as tile
import concourse.mybir as mybir
from concourse.bass_utils import run_bass_kernel_spmd

F32 = mybir.dt.float32
BF16 = mybir.dt.bfloat16
NPBF = np.dtype(ml_dtypes.bfloat16)
AF = mybir.ActivationFunctionType

B = 2
T = 4096
D = 768
H = 12
DK = 64
NCORES = 8
HL = 3  # heads per core
ND = D // 128  # 6 d-tiles
NKT = T // 128  # 32 k-tiles
NQB = T // 512  # 8 q-blocks
NTSB = T // 512  # 8 t-superblocks

_CACHE = {}


def _emit(tc):
    nc = tc.nc
    xT_d = nc.dram_tensor("xT", [D, T], BF16, kind="ExternalInput").ap()
    wqk_d = nc.dram_tensor("wqkT", [D, 384], BF16, kind="ExternalInput").ap()
    wv_d = nc.dram_tensor("wvT", [D, HL * DK], BF16, kind="ExternalInput").ap()
    wo_d = nc.dram_tensor("woT", [HL, DK, D], BF16, kind="ExternalInput").ap()
    y_d = nc.dram_tensor("yT", [D, T], F32, kind="ExternalOutput").ap()

    ctx = ExitStack()
    const = ctx.enter_context(tc.tile_pool(name="const", bufs=1))
    persist = ctx.enter_context(tc.tile_pool(name="persist", bufs=1))
    ptpool = ctx.enter_context(tc.tile_pool(name="pt", bufs=6))
    spool = ctx.enter_context(tc.tile_pool(name="sp", bufs=2))
    ypool = ctx.enter_context(tc.tile_pool(name="yp", bufs=2))
    # PSUM (8 banks): psA 2x[128,1024]f32 = 4 for score tiles; psB 2x1 for
    # AV accumulators; psC 2x1 shared by projection / out-proj fillers.
    psA = ctx.enter_context(tc.tile_pool(name="psA", bufs=2, space="PSUM"))
    psB = ctx.enter_context(tc.tile_pool(name="psB", bufs=2, space="PSUM"))
    psC = ctx.enter_context(tc.tile_pool(name="psC", bufs=2, space="PSUM"))

    # ---- constants ----
    # triangle mask for the first 128 valid columns of each diagonal band
    # tile: tri[k, j] = 1 for j >= k else 0
    tri = const.tile([128, 128], BF16, name="tri")
    nc.gpsimd.memset(tri, 1.0)
    nc.gpsimd.affine_select(
        out=tri, in_=tri, compare_op=mybir.AluOpType.is_ge, fill=0.0,
        base=0, pattern=[[1, 128]], channel_multiplier=-1,
    )

    wqk_sb = const.tile([128, ND, 384], BF16)
    nc.sync.dma_start(out=wqk_sb, in_=wqk_d.rearrange("(j p) e -> p j e", p=128))
    # wv/wo tiles are created here but their DMAs are deferred into the
    # warm-up sequence so the first QK matmul's operands transfer first
    wv_sb = const.tile([128, ND, HL * DK], BF16)
    wo01_sb = const.tile([128, D], BF16)  # head0 rows on 0:64, head1 on 64:128
    wo2_sb = const.tile([DK, D], BF16)

    def load_w_late():
        nc.sync.dma_start(out=wv_sb, in_=wv_d.rearrange("(j p) e -> p j e", p=128))
        nc.sync.dma_start(out=wo01_sb, in_=wo_d[0:2].rearrange("h p d -> (h p) d"))
        nc.sync.dma_start(out=wo2_sb, in_=wo_d[2])

    # ---- persistent activations ----
    xT_sb = persist.tile([128, ND, T], BF16, name="xT")
    # KA: [K^T_h0 ; K^T_h1], QB: [Q^T_h0 ; Q^T_h1] on partition halves
    KA = persist.tile([128, T], BF16, name="KA")
    QB = persist.tile([128, T], BF16, name="QB")
    C2 = persist.tile([128, T], BF16, name="C2")  # [K^T_h2 ; Q^T_h2]
    D2 = persist.tile([128, T], BF16, name="D2")  # [Q^T_h2 ; K^T_h2] (swapped)
    V = persist.tile([128, HL, NKT, DK + 1], BF16, name="V")
    nc.gpsimd.memset(V[:, :, :, DK : DK + 1], 1.0)  # ones col -> softmax sums
    # out^T staging, double-buffered by q-block parity (out-proj is deferred
    # into the next q-block's filler slots)
    ot01 = [persist.tile([128, 512], BF16, name=f"ot01_{p}") for p in range(2)]
    ot2 = [persist.tile([DK, 512], BF16, name=f"ot2_{p}") for p in range(2)]

    qk_dest = [KA, QB, C2]
    xT_dr = xT_d.rearrange("(j p) t -> p j t", p=128)

    # ---- filler items: projection work for one t-superblock ----
    def proj_fillers(tsb):
        blk = slice(tsb * 512, (tsb + 1) * 512)

        def dma_item():
            if tsb == 0:
                # split per d-tile so the first QK matmul chain can start
                # as soon as its first operand stripe lands
                for dj in range(ND):
                    nc.sync.dma_start(
                        out=xT_sb[:, dj, blk], in_=xT_dr[:, dj, blk]
                    )
                load_w_late()
            else:
                nc.sync.dma_start(out=xT_sb[:, :, blk], in_=xT_dr[:, :, blk])

        def qk_item(et):
            ps_q = psC.tile([128, 512], F32, name="ps_q", tag="pc")
            e0 = et * 128
            for dj in range(ND):
                nc.tensor.matmul(
                    ps_q,
                    lhsT=wqk_sb[:, dj, e0 : e0 + 128],
                    rhs=xT_sb[:, dj, blk],
                    start=(dj == 0), stop=(dj == ND - 1),
                )
            nc.vector.tensor_copy(qk_dest[et][:, blk], ps_q)
            if et == 2:
                # D2 = partition-swapped copy of C2 (h2 self-pairing)
                nc.sync.dma_start(out=D2[0:64, blk], in_=C2[64:128, blk])
                nc.sync.dma_start(out=D2[64:128, blk], in_=C2[0:64, blk])

        def v_item(tt):
            t0 = (tsb * 4 + tt) * 128
            ps_v = psC.tile([128, HL * DK], F32, name="ps_v", tag="pc")
            for dj in range(ND):
                nc.tensor.matmul(
                    ps_v,
                    lhsT=xT_sb[:, dj, t0 : t0 + 128],
                    rhs=wv_sb[:, dj, :],
                    start=(dj == 0), stop=(dj == ND - 1),
                )
            kt = tsb * 4 + tt
            nc.vector.tensor_copy(
                V[:, :, kt, 0:DK], ps_v.rearrange("p (h c) -> p h c", h=HL)
            )

        items = [dma_item]
        items += [lambda et=et: qk_item(et) for et in range(3)]
        items += [lambda tt=tt: v_item(tt) for tt in range(4)]
        return items

    # ---- filler items: out-projection of one q-block ----
    def outproj_fillers(qb):
        qblk = slice(qb * 512, (qb + 1) * 512)
        o01, o2 = ot01[qb % 2], ot2[qb % 2]

        def y_item(dj):
            dblk = slice(dj * 128, (dj + 1) * 128)
            ps_y = psC.tile([128, 512], F32, name="ps_y", tag="pc")
            nc.tensor.matmul(
                ps_y, lhsT=wo01_sb[:, dblk], rhs=o01,
                start=True, stop=False, skip_group_check=True,
            )
            nc.tensor.matmul(
                ps_y, lhsT=wo2_sb[:, dblk], rhs=o2,
                start=False, stop=True, skip_group_check=True,
            )
            y_sb = ypool.tile([128, 512], F32, name="y_sb")
            nc.vector.tensor_copy(y_sb, ps_y)
            nc.sync.dma_start(out=y_d[dblk, qblk], in_=y_sb)

        return [lambda dj=dj: y_item(dj) for dj in range(ND)]

    # fq_proj: hard deadline (drained before the attention block that reads
    # it); fq_out: deferred out-proj, drains opportunistically.
    fq_proj = deque()
    fq_out = deque()

    def emit_fillers(n):
        for _ in range(n):
            if fq_proj:
                fq_proj.popleft()()
            elif fq_out:
                fq_out.popleft()()
            else:
                return

    # warm-up: project t-superblock 0 before attention starts
    for it in proj_fillers(0):
        it()
    fq_proj.extend(proj_fillers(1))

    # ================= attention, pipelined =================
    for qb in range(NQB):
        nk = 4 * (qb + 1)
        o01, o2 = ot01[qb % 2], ot2[qb % 2]
        # k-pair list; the two diagonal-band pairs are reordered (hi, lo)
        # so the exp range stays a single contiguous span per pt tile
        pairs = [(2 * kp, 2 * kp + 1) for kp in range(nk // 2)]
        pairs[-2] = (nk - 3, nk - 4)
        pairs[-1] = (nk - 1, nk - 2)
        for hpass, heads in enumerate([(0, 1), (2,)]):
            psav = {h: psB.tile([DK + 1, 512], F32, name=f"psav{h}", tag="pb")
                    for h in heads}
            for kt_a, kt_b in pairs:
                ss = {h: psA.tile([128, 1024], F32, name=f"ss{h}", tag="pa")
                      for h in heads}
                lo_a = max(0, kt_a - 4 * qb) * 128  # valid col start (band)
                lo_b = max(0, kt_b - 4 * qb) * 128
                for i, (kt, lo) in enumerate(((kt_a, lo_a), (kt_b, lo_b))):
                    kblk = slice(kt * 128, (kt + 1) * 128)
                    dst = slice(i * 512 + lo, i * 512 + 512)
                    qsub = slice(qb * 512 + lo, (qb + 1) * 512)
                    if hpass == 0:
                        nc.tensor.matmul(
                            ss[0][:, dst], lhsT=KA[0:64, kblk],
                            rhs=QB[0:64, qsub], start=True, stop=True,
                        )
                        nc.tensor.matmul(
                            ss[1][:, dst], lhsT=KA[64:128, kblk],
                            rhs=QB[64:128, qsub], start=True, stop=True,
                        )
                    elif i == 0:
                        nc.tensor.matmul(
                            ss[2][:, dst], lhsT=C2[0:64, kblk],
                            rhs=D2[0:64, qsub], start=True, stop=True,
                        )
                    else:
                        nc.tensor.matmul(
                            ss[2][:, dst], lhsT=D2[64:128, kblk],
                            rhs=C2[64:128, qsub], start=True, stop=True,
                        )
                emit_fillers(1)
                for h in heads:
                    pt = ptpool.tile([128, 1024], BF16, name="pt")
                    # one contiguous exp span [lo_a:1024]; for a reordered
                    # band pair (hi, lo) any gap columns hold junk that the
                    # AV rhs slices below never touch
                    nc.scalar.activation(
                        pt[:, lo_a:1024], ss[h][:, lo_a:1024], AF.Exp,
                        scale=0.125,
                    )
                    # triangle mask on the first 128 valid cols of band tiles
                    for i, (kt, lo) in enumerate(((kt_a, lo_a), (kt_b, lo_b))):
                        if kt >= 4 * qb:
                            c0 = i * 512 + lo
                            nc.vector.tensor_mul(
                                pt[:, c0 : c0 + 128], pt[:, c0 : c0 + 128], tri
                            )
                    # AV ascending kt within the pair (kt==0 carries the
                    # full-width start=True that initializes the bank)
                    for i, kt, lo in sorted(
                        ((0, kt_a, lo_a), (1, kt_b, lo_b)), key=lambda e: e[1]
                    ):
                        nc.tensor.matmul(
                            psav[h][:, lo:512],
                            lhsT=V[:, h, kt, :],
                            rhs=pt[:, i * 512 + lo : i * 512 + 512],
                            start=(kt == 0), stop=(kt == nk - 1),
                            skip_group_check=True,
                        )
                if len(fq_proj) + len(fq_out) > 10:
                    emit_fillers(2)
            # normalize: out^T = psav rows 0:64 / sums (row 64); copy psav
            # to SBUF right away so the psum bank recycles quickly
            for h in heads:
                av_sb = spool.tile([DK, 512], F32, name="av_sb", tag="av")
                nc.vector.tensor_copy(av_sb, psav[h][0:DK, :])
                sums_sb = spool.tile([1, 512], F32, name="sums_sb", tag="sm")
                nc.vector.tensor_copy(sums_sb, psav[h][DK : DK + 1, :])
                chop = spool.tile([128, 4], F32, name="chop", tag="ch")
                nc.sync.dma_start(out=chop, in_=sums_sb)
                recipC = spool.tile([128, 4], F32, name="recipC", tag="rc")
                nc.vector.reciprocal(recipC, chop)
                recipR = spool.tile([1, 512], F32, name="recipR", tag="rr")
                nc.sync.dma_start(out=recipR, in_=recipC)
                recipb = spool.tile([DK, 512], F32, name="recipb", tag="rb")
                nc.gpsimd.partition_broadcast(recipb, recipR, channels=DK)
                if h == 0:
                    nc.vector.tensor_mul(o01[0:DK, :], av_sb, recipb)
                elif h == 1:
                    ot1s = spool.tile([DK, 512], BF16, name="ot1s", tag="o1")
                    nc.vector.tensor_mul(ot1s, av_sb, recipb)
                    nc.sync.dma_start(out=o01[DK:128, :], in_=ot1s)
                else:
                    nc.vector.tensor_mul(o2, av_sb, recipb)
        # hard deadline: projections for the next q-block must be fully
        # emitted before its attention reads KA/QB/C2/D2/V
        emit_n = len(fq_proj)
        for _ in range(emit_n):
            fq_proj.popleft()()
        # defer this q-block's out-projection into upcoming filler slots
        fq_out.extend(outproj_fillers(qb))
        if qb + 2 < NTSB:
            fq_proj.extend(proj_fillers(qb + 2))
    while fq_out:
        fq_out.popleft()()
    ctx.close()


def build():
    if "nc" in _CACHE:
        return _CACHE["nc"]
    nc = bacc.Bacc(
        "TRN2", target_bir_lowering=False, debug=False, num_devices=NCORES
    )
    with tile.TileContext(nc) as tc:
        _emit(tc)
    nc.compile()
    _CACHE["nc"] = nc
    return nc


def make_in_maps(x, w_qkv, w_out):
    x = np.asarray(x, dtype=np.float32)
    w_qkv = np.asarray(w_qkv, dtype=np.float32)
    w_out = np.asarray(w_out, dtype=np.float32)
    wq = w_qkv[0:D]        # [768, 768], rows = q features
    wk = w_qkv[D : 2 * D]
    wv = w_qkv[2 * D :]
    xT = [np.ascontiguousarray(x[b].T).astype(NPBF) for b in range(B)]
    in_maps = []
    for c in range(NCORES):
        b, g = divmod(c, 4)
        hs = [3 * g + j for j in range(HL)]  # global head ids
        h0, h1, h2 = hs
        cols = []
        for pair in ((wk, h0), (wk, h1), (wq, h0), (wq, h1), (wk, h2), (wq, h2)):
            w, h = pair
            cols.append(w[h * DK : (h + 1) * DK].T)  # [768, 64]
        wqkT = np.ascontiguousarray(np.concatenate(cols, axis=1)).astype(NPBF)
        wvT = np.ascontiguousarray(
            np.concatenate([wv[h * DK : (h + 1) * DK].T for h in hs], axis=1)
        ).astype(NPBF)  # [768, 192]
        woT = np.ascontiguousarray(
            np.stack([w_out[:, h * DK : (h + 1) * DK].T for h in hs])
        ).astype(NPBF)  # [3, 64, 768]
        in_maps.append(
            {
                "xT": xT[b],
                "wqkT": wqkT,
                "wvT": wvT,
                "woT": woT,
            }
        )
    return in_maps


def run(inputs, trace=False):
    """Run on hardware; returns (y [B,T,D] fp32, BassKernelResults)."""
    nc = build()
    in_maps = make_in_maps(inputs["x"], inputs["w_qkv"], inputs["w_out"])
    br = run_bass_kernel_spmd(nc, in_maps, list(range(NCORES)), trace=trace)
    y = np.zeros((B, T, D), dtype=np.float32)
    for c in range(NCORES):
        b = c // 4
        y[b] += np.asarray(br.results[c]["yT"], dtype=np.float32).T
    return y, br


def kernel(x, w_qkv, w_out):
    y, _ = run({"x": x, "w_qkv": w_qkv, "w_out": w_out})
    return y


# revision 25
# speedup vs baseline: 1.2544x; 1.0144x over previous
"""Multi-head causal self-attention (B=2, T=4096, D=768, H=12) on 8 trn2 cores.

Sharding: core c -> batch b = c//4, heads 3*(c%4) .. 3*(c%4)+2.
qkv_proj column-parallel (each core computes Q/K/V only for its heads),
out_proj row-parallel (each core emits a partial y^T; host sums the 4
partials per batch).

v3: bf16 matmuls + host-side x^T; software-pipelined schedule: projection
and out-proj matmuls are injected as PE "filler" work between attention
k-pair steps so the tensor engine never idles (keeps the DVFS p-state at
max); causal band tiles trim their invalid columns from the scores/exp/AV
work (single 128x128 triangle mask replaces the wide band masks); AV psum
accumulators are copied to SBUF immediately so their banks recycle fast.

Device dataflow:
  x^T bf16 DMA'd per 512-col chunk -> Q^T/K^T via transposed projection
  (W^T stationary, x^T streaming) -> V natural (x^T chunks stationary,
  Wv^T streaming) -> S^T = K Q^T in [k,q] layout, two heads row-paired on
  opposite PE halves -> exp on ScalarE writing bf16 -> triangle masks on
  DVE for diagonal tiles -> out^T = V^T P^T with a ones column appended
  to V collecting softmax denominators in psum row 64 -> normalize via
  batched reciprocal + gpsimd partition broadcast (double-buffered by qb
  parity) -> y^T = Wo^T.T out^T, deferred one q-block as filler work.
"""

import sys

sys.path.insert(0, "/opt/trn_rl_repo")

import numpy as np
import ml_dtypes
from collections import deque
from contextlib import ExitStack

import concourse.bass as bass
import concourse.bacc as bacc
import concourse.tile as tile
import concourse.mybir as mybir
from concourse.bass_utils import run_bass_kernel_spmd

F32 = mybir.dt.float32
BF16 = mybir.dt.bfloat16
NPBF = np.dtype(ml_dtypes.bfloat16)
AF = mybir.ActivationFunctionType

B = 2
T = 4096
D = 768
H = 12
DK = 64
NCORES = 8
HL = 3  # heads per core
ND = D // 128  # 6 d-tiles
NKT = T // 128  # 32 k-tiles
NQB = T // 512  # 8 q-blocks
NTSB = T // 512  # 8 t-superblocks

_CACHE = {}


def _emit(tc):
    nc = tc.nc
    xT_d = nc.dram_tensor("xT", [D, T], BF16, kind="ExternalInput").ap()
    wqk_d = nc.dram_tensor("wqkT", [D, 384], BF16, kind="ExternalInput").ap()
    wv_d = nc.dram_tensor("wvT", [D, HL * DK], BF16, kind="ExternalInput").ap()
    wo_d = nc.dram_tensor("woT", [HL, DK, D], BF16, kind="ExternalInput").ap()
    y_d = nc.dram_tensor("yT", [D, T], F32, kind="ExternalOutput").ap()

    ctx = ExitStack()
    const = ctx.enter_context(tc.tile_pool(name="const", bufs=1))
    persist = ctx.enter_context(tc.tile_pool(name="persist", bufs=1))
    ptpool = ctx.enter_context(tc.tile_pool(name="pt", bufs=8))
    spool = ctx.enter_context(tc.tile_pool(name="sp", bufs=2))
    ypool = ctx.enter_context(tc.tile_pool(name="yp", bufs=2))
    # PSUM (8 banks): psA 2x[128,1024]f32 = 4 for score tiles; psB 2x1 for
    # AV accumulators; psC 2x1 shared by projection / out-proj fillers.
    psA = ctx.enter_context(tc.tile_pool(name="psA", bufs=2, space="PSUM"))
    psB = ctx.enter_context(tc.tile_pool(name="psB", bufs=2, space="PSUM"))
    psC = ctx.enter_context(tc.tile_pool(name="psC", bufs=2, space="PSUM"))

    # ---- constants ----
    # triangle mask for the first 128 valid columns of each diagonal band
    # tile: tri[k, j] = 1 for j >= k else 0
    tri = const.tile([128, 128], BF16, name="tri")
    nc.gpsimd.memset(tri, 1.0)
    nc.gpsimd.affine_select(
        out=tri, in_=tri, compare_op=mybir.AluOpType.is_ge, fill=0.0,
        base=0, pattern=[[1, 128]], channel_multiplier=-1,
    )

    # weight tiles are created here but their DMAs are deferred into the
    # warm-up sequence so the first QK matmul's operands transfer first
    wqk_sb = const.tile([128, ND, 384], BF16)
    wqk_dr = wqk_d.rearrange("(j p) e -> p j e", p=128)
    wv_sb = const.tile([128, ND, HL * DK], BF16)
    wo01_sb = const.tile([128, D], BF16)  # head0 rows on 0:64, head1 on 64:128
    wo2_sb = const.tile([DK, D], BF16)

    def load_w_late():
        nc.sync.dma_start(out=wv_sb, in_=wv_d.rearrange("(j p) e -> p j e", p=128))
        nc.sync.dma_start(out=wo01_sb, in_=wo_d[0:2].rearrange("h p d -> (h p) d"))
        nc.sync.dma_start(out=wo2_sb, in_=wo_d[2])

    # ---- persistent activations ----
    xT_sb = persist.tile([128, ND, T], BF16, name="xT")
    # KA: [K^T_h0 ; K^T_h1], QB: [Q^T_h0 ; Q^T_h1] on partition halves
    KA = persist.tile([128, T], BF16, name="KA")
    QB = persist.tile([128, T], BF16, name="QB")
    C2 = persist.tile([128, T], BF16, name="C2")  # [K^T_h2 ; Q^T_h2]
    D2 = persist.tile([128, T], BF16, name="D2")  # [Q^T_h2 ; K^T_h2] (swapped)
    V = persist.tile([128, HL, NKT, DK + 1], BF16, name="V")
    nc.gpsimd.memset(V[:, :, :, DK : DK + 1], 1.0)  # ones col -> softmax sums
    # out^T staging, double-buffered by q-block parity (out-proj is deferred
    # into the next q-block's filler slots)
    ot01 = [persist.tile([128, 512], BF16, name=f"ot01_{p}") for p in range(2)]
    ot2 = [persist.tile([DK, 512], BF16, name=f"ot2_{p}") for p in range(2)]

    qk_dest = [KA, QB, C2]
    xT_dr = xT_d.rearrange("(j p) t -> p j t", p=128)

    # ---- filler items: projection work for one t-superblock ----
    def proj_fillers(tsb):
        blk = slice(tsb * 512, (tsb + 1) * 512)

        def dma_item():
            if tsb == 0:
                # split per d-tile so the first QK matmul chain can start
                # as soon as its first operand stripes land
                for dj in range(ND):
                    nc.sync.dma_start(
                        out=wqk_sb[:, dj, :], in_=wqk_dr[:, dj, :]
                    )
                    nc.sync.dma_start(
                        out=xT_sb[:, dj, blk], in_=xT_dr[:, dj, blk]
                    )
                load_w_late()
            else:
                nc.sync.dma_start(out=xT_sb[:, :, blk], in_=xT_dr[:, :, blk])

        def qk_item(et):
            ps_q = psC.tile([128, 512], F32, name="ps_q", tag="pc")
            e0 = et * 128
            for dj in range(ND):
                nc.tensor.matmul(
                    ps_q,
                    lhsT=wqk_sb[:, dj, e0 : e0 + 128],
                    rhs=xT_sb[:, dj, blk],
                    start=(dj == 0), stop=(dj == ND - 1),
                )
            nc.vector.tensor_copy(qk_dest[et][:, blk], ps_q)
            if et == 2:
                # D2 = partition-swapped copy of C2 (h2 self-pairing)
                nc.sync.dma_start(out=D2[0:64, blk], in_=C2[64:128, blk])
                nc.sync.dma_start(out=D2[64:128, blk], in_=C2[0:64, blk])

        def v_item(tt):
            t0 = (tsb * 4 + tt) * 128
            ps_v = psC.tile([128, HL * DK], F32, name="ps_v", tag="pc")
            for dj in range(ND):
                nc.tensor.matmul(
                    ps_v,
                    lhsT=xT_sb[:, dj, t0 : t0 + 128],
                    rhs=wv_sb[:, dj, :],
                    start=(dj == 0), stop=(dj == ND - 1),
                )
            kt = tsb * 4 + tt
            nc.vector.tensor_copy(
                V[:, :, kt, 0:DK], ps_v.rearrange("p (h c) -> p h c", h=HL)
            )

        items = [dma_item]
        items += [lambda et=et: qk_item(et) for et in range(3)]
        items += [lambda tt=tt: v_item(tt) for tt in range(4)]
        return items

    # ---- filler items: out-projection of one q-block ----
    def outproj_fillers(qb):
        qblk = slice(qb * 512, (qb + 1) * 512)
        o01, o2 = ot01[qb % 2], ot2[qb % 2]

        def y_item(dj):
            dblk = slice(dj * 128, (dj + 1) * 128)
            ps_y = psC.tile([128, 512], F32, name="ps_y", tag="pc")
            nc.tensor.matmul(
                ps_y, lhsT=wo01_sb[:, dblk], rhs=o01,
                start=True, stop=False, skip_group_check=True,
            )
            nc.tensor.matmul(
                ps_y, lhsT=wo2_sb[:, dblk], rhs=o2,
                start=False, stop=True, skip_group_check=True,
            )
            y_sb = ypool.tile([128, 512], F32, name="y_sb")
            nc.vector.tensor_copy(y_sb, ps_y)
            nc.sync.dma_start(out=y_d[dblk, qblk], in_=y_sb)

        return [lambda dj=dj: y_item(dj) for dj in range(ND)]

    # fq_proj: hard deadline (drained before the attention block that reads
    # it); fq_out: deferred out-proj, drains opportunistically.
    fq_proj = deque()
    fq_out = deque()

    def emit_fillers(n):
        for _ in range(n):
            if fq_proj:
                fq_proj.popleft()()
            elif fq_out:
                fq_out.popleft()()
            else:
                return

    # warm-up: project t-superblock 0 before attention starts
    for it in proj_fillers(0):
        it()
    fq_proj.extend(proj_fillers(1))

    # ================= attention, pipelined =================
    for qb in range(NQB):
        nk = 4 * (qb + 1)
        o01, o2 = ot01[qb % 2], ot2[qb % 2]
        # k-pair list; the two diagonal-band pairs are reordered (hi, lo)
        # so the exp range stays a single contiguous span per pt tile
        pairs = [(2 * kp, 2 * kp + 1) for kp in range(nk // 2)]
        pairs[-2] = (nk - 3, nk - 4)
        pairs[-1] = (nk - 1, nk - 2)
        for hpass, heads in enumerate([(0, 1), (2,)]):
            psav = {h: psB.tile([DK + 1, 512], F32, name=f"psav{h}", tag="pb")
                    for h in heads}
            # AV matmuls for pair p are emitted during pair p+1, after its
            # scores: by then exp(p) has finished, so the PE never waits on
            # the scalar engine (depth-1 software pipeline).
            pend_av = None
            for kt_a, kt_b in pairs:
                ss = {h: psA.tile([128, 1024], F32, name=f"ss{h}", tag="pa")
                      for h in heads}
                lo_a = max(0, kt_a - 4 * qb) * 128  # valid col start (band)
                lo_b = max(0, kt_b - 4 * qb) * 128
                for i, (kt, lo) in enumerate(((kt_a, lo_a), (kt_b, lo_b))):
                    kblk = slice(kt * 128, (kt + 1) * 128)
                    dst = slice(i * 512 + lo, i * 512 + 512)
                    qsub = slice(qb * 512 + lo, (qb + 1) * 512)
                    if hpass == 0:
                        nc.tensor.matmul(
                            ss[0][:, dst], lhsT=KA[0:64, kblk],
                            rhs=QB[0:64, qsub], start=True, stop=True,
                        )
                        nc.tensor.matmul(
                            ss[1][:, dst], lhsT=KA[64:128, kblk],
                            rhs=QB[64:128, qsub], start=True, stop=True,
                        )
                    elif i == 0:
                        nc.tensor.matmul(
                            ss[2][:, dst], lhsT=C2[0:64, kblk],
                            rhs=D2[0:64, qsub], start=True, stop=True,
                        )
                    else:
                        nc.tensor.matmul(
                            ss[2][:, dst], lhsT=D2[64:128, kblk],
                            rhs=C2[64:128, qsub], start=True, stop=True,
                        )
                emit_fillers(1)
                if pend_av is not None:
                    pend_av()
                pts = {}
                for h in heads:
                    pt = ptpool.tile([128, 1024], BF16, name="pt")
                    pts[h] = pt
                    # one contiguous exp span [lo_a:1024]; for a reordered
                    # band pair (hi, lo) any gap columns hold junk that the
                    # AV rhs slices below never touch
                    nc.scalar.activation(
                        pt[:, lo_a:1024], ss[h][:, lo_a:1024], AF.Exp,
                        scale=0.125,
                    )
                    # triangle mask on the first 128 valid cols of band tiles
                    for i, (kt, lo) in enumerate(((kt_a, lo_a), (kt_b, lo_b))):
                        if kt >= 4 * qb:
                            c0 = i * 512 + lo
                            nc.vector.tensor_mul(
                                pt[:, c0 : c0 + 128], pt[:, c0 : c0 + 128], tri
                            )

                def make_av(pts=pts, kt_a=kt_a, kt_b=kt_b, lo_a=lo_a, lo_b=lo_b):
                    # AV ascending kt within the pair (kt==0 carries the
                    # full-width start=True that initializes the bank)
                    for h, pt in pts.items():
                        for i, kt, lo in sorted(
                            ((0, kt_a, lo_a), (1, kt_b, lo_b)),
                            key=lambda e: e[1],
                        ):
                            nc.tensor.matmul(
                                psav[h][:, lo:512],
                                lhsT=V[:, h, kt, :],
                                rhs=pt[:, i * 512 + lo : i * 512 + 512],
                                start=(kt == 0), stop=(kt == nk - 1),
                                skip_group_check=True,
                            )

                pend_av = make_av
                if len(fq_proj) + len(fq_out) > 10:
                    emit_fillers(2)
            pend_av()
            # normalize: out^T = psav rows 0:64 / sums (row 64); copy psav
            # to SBUF right away so the psum bank recycles quickly
            for h in heads:
                av_sb = spool.tile([DK, 512], F32, name="av_sb", tag="av")
                nc.vector.tensor_copy(av_sb, psav[h][0:DK, :])
                sums_sb = spool.tile([1, 512], F32, name="sums_sb", tag="sm")
                nc.vector.tensor_copy(sums_sb, psav[h][DK : DK + 1, :])
                chop = spool.tile([128, 4], F32, name="chop", tag="ch")
                nc.sync.dma_start(out=chop, in_=sums_sb)
                recipC = spool.tile([128, 4], F32, name="recipC", tag="rc")
                nc.vector.reciprocal(recipC, chop)
                recipR = spool.tile([1, 512], F32, name="recipR", tag="rr")
                nc.sync.dma_start(out=recipR, in_=recipC)
                recipb = spool.tile([DK, 512], F32, name="recipb", tag="rb")
                nc.gpsimd.partition_broadcast(recipb, recipR, channels=DK)
                if h == 0:
                    nc.vector.tensor_mul(o01[0:DK, :], av_sb, recipb)
                elif h == 1:
                    ot1s = spool.tile([DK, 512], BF16, name="ot1s", tag="o1")
                    nc.vector.tensor_mul(ot1s, av_sb, recipb)
                    nc.sync.dma_start(out=o01[DK:128, :], in_=ot1s)
                else:
                    nc.vector.tensor_mul(o2, av_sb, recipb)
        # hard deadline: projections for the next q-block must be fully
        # emitted before its attention reads KA/QB/C2/D2/V
        emit_n = len(fq_proj)
        for _ in range(emit_n):
            fq_proj.popleft()()
        # defer this q-block's out-projection into upcoming filler slots
        fq_out.extend(outproj_fillers(qb))
        if qb + 2 < NTSB:
            fq_proj.extend(proj_fillers(qb + 2))
    while fq_out:
        fq_out.popleft()()
    ctx.close()


def build():
    if "nc" in _CACHE:
        return _CACHE["nc"]
    nc = bacc.Bacc(
        "TRN2", target_bir_lowering=False, debug=False, num_devices=NCORES
    )
    with tile.TileContext(nc) as tc:
        _emit(tc)
    nc.compile()
    _CACHE["nc"] = nc
    return nc


def make_in_maps(x, w_qkv, w_out):
    x = np.asarray(x, dtype=np.float32)
    w_qkv = np.asarray(w_qkv, dtype=np.float32)
    w_out = np.asarray(w_out, dtype=np.float32)
    wq = w_qkv[0:D]        # [768, 768], rows = q features
    wk = w_qkv[D : 2 * D]
    wv = w_qkv[2 * D :]
    xT = [np.ascontiguousarray(x[b].T).astype(NPBF) for b in range(B)]
    in_maps = []
    for c in range(NCORES):
        b, g = divmod(c, 4)
        hs = [3 * g + j for j in range(HL)]  # global head ids
        h0, h1, h2 = hs
        cols = []
        for pair in ((wk, h0), (wk, h1), (wq, h0), (wq, h1), (wk, h2), (wq, h2)):
            w, h = pair
            cols.append(w[h * DK : (h + 1) * DK].T)  # [768, 64]
        wqkT = np.ascontiguousarray(np.concatenate(cols, axis=1)).astype(NPBF)
        wvT = np.ascontiguousarray(
            np.concatenate([wv[h * DK : (h + 1) * DK].T for h in hs], axis=1)
        ).astype(NPBF)  # [768, 192]
        woT = np.ascontiguousarray(
            np.stack([w_out[:, h * DK : (h + 1) * DK].T for h in hs])
        ).astype(NPBF)  # [3, 64, 768]
        in_maps.append(
            {
                "xT": xT[b],
                "wqkT": wqkT,
                "wvT": wvT,
                "woT": woT,
            }
        )
    return in_maps


def run(inputs, trace=False):
    """Run on hardware; returns (y [B,T,D] fp32, BassKernelResults)."""
    nc = build()
    in_maps = make_in_maps(inputs["x"], inputs["w_qkv"], inputs["w_out"])
    br = run_bass_kernel_spmd(nc, in_maps, list(range(NCORES)), trace=trace)
    y = np.zeros((B, T, D), dtype=np.float32)
    for c in range(NCORES):
        b = c // 4
        y[b] += np.asarray(br.results[c]["yT"], dtype=np.float32).T
    return y, br


def kernel(x, w_qkv, w_out):
    y, _ = run({"x": x, "w_qkv": w_qkv, "w_out": w_out})
    return y
